# revision 1
# baseline (speedup 1.0000x reference)
"""MBConv block with MoE routing on 8 trn2 cores — fp8 DoubleRow everywhere.

Sharding: pure data parallel — batch 64 split 8 samples per core; all weights
replicated. Device kernel computes routing, expert-weight aggregation, expand
conv (fp8 DoubleRow over channel halves + a ones-channel carrying bn1's bias),
per-sample depthwise conv as fp8e4 DoubleRow diagonal matmuls (2 taps per
matmul, even-stride pairs, plus a bias pair against a ones strip carrying
bn2's bias), squeeze-excitation, pointwise projection (fp8 DoubleRow over
chunk pairs), bn3 fold and residual.

All BN folds are absorbed into weights (a1 -> expand cols, a2 -> depthwise
kernels, biases via ones channels) so every activation instruction uses only
immediate scale/bias — per-partition AP scale/bias activations are ~3x slower
on hardware.

Numerics: expand weights x8, depthwise kernels x64, pointwise weights x64 —
each pre-scale undone by the activation's immediate scale or the bn3 fold.

Self-contained: hardcodes all shapes; host side only reshapes/prepacks weights.
"""

import os
import sys
import time

for _p in ("/opt/trn_rl_repo", os.path.expanduser("~/.axon_site/_ro/trn_rl_repo")):
    if os.path.isdir(_p) and _p not in sys.path:
        sys.path.insert(0, _p)

import contextlib

import numpy as np

import concourse.bacc as bacc
import concourse.bass as bass
import concourse.tile as tile
from concourse import mybir

F32 = mybir.dt.float32
BF16 = mybir.dt.bfloat16
FP8 = mybir.dt.float8e4
AF = mybir.ActivationFunctionType
ALU = mybir.AluOpType
AX = mybir.AxisListType
DR = mybir.MatmulPerfMode.DoubleRow

# dims (must match the problem spec)
B, CIN, H, W = 64, 96, 28, 28
NCORES = 8
BL = B // NCORES          # 8 samples per core
E = 4
HID = 576
KK = 5
T = KK * KK               # 25 taps
TP = 26                   # taps padded with one zero tap for DoubleRow pairing
RED = 24                  # SE reduced dim
RHID = 24                 # routing hidden
COUT = 96
EPS = 1e-3
HW = H * W                # 784
NG = 5                    # ceil(576/128) channel chunks
GP = 128
HIDP = NG * GP            # 640 padded
PW = 32                   # padded row stride
NH = 2                    # output row halves (14 rows each)
RH = H // NH              # 14
NF = RH * W               # 392 free elems per half
XPW = 1036                # padded x1 tile width (68 head + 28*32 + tail)
KSCALE = 64.0             # fp8 kernel pre-scale, undone by bn2 act imm scale
ESCALE = 8.0              # fp8 expand pre-scale, undone by bn1 act imm scale
PSCALE = 64.0             # fp8 pointwise pre-scale, folded out via a3
CH = CIN // 2 + 1         # 49: expand DR contraction half + ones channel

# DoubleRow tap pairs (t0, t1): the rhs plane stride (byte delta between the
# two shifted windows) must be EVEN — odd strides crash the DGE. Within-row
# pairs are taken 2 apart (stride 2), the kw=4 column pairs vertically
# (stride 32), tap 24 pairs with zero tap 25, and the bias pair (26, 27)
# reads a ones strip against diag(64*b2) + a zero plane.
DW_PAIRS = (
    [(5 * r, 5 * r + 2) for r in range(KK)]
    + [(5 * r + 1, 5 * r + 3) for r in range(KK)]
    + [(4, 9), (14, 19), (24, 25)]
)

# diag-slab build split: (engine_name, g, tap_lo, tap_hi). All DVE: the
# build is a single scalar_tensor_tensor per set — (kern_f32_bcast x 64)
# masked by a packed replicated-identity fp8 constant — which runs ~2x
# faster than the tensor_tensor formulation and ~4x faster than Pool/Act.
DIAG_SPLIT = [
    ("dve", 0, 0, TP), ("dve", 1, 0, TP), ("dve", 2, 0, TP),
    ("dve", 3, 0, TP), ("dve", 4, 0, TP),
]


def _tap_off(t):
    kh, kw = divmod(t, KK)
    return PW * kh + kw


def _build_program(reps=1, ablate=()):
    nc = bacc.Bacc(None, target_bir_lowering=False)

    dt = lambda name, shape: nc.dram_tensor(name, shape, F32, kind="ExternalInput")
    x_d = dt("x", [CIN, BL, HW])
    x8_d = nc.dram_tensor("x8", [CH, 2, BL, HW], FP8, kind="ExternalInput")
    exp8_d = nc.dram_tensor("exp8", [CH, 2, HIDP], FP8, kind="ExternalInput")
    identrep_d = nc.dram_tensor("identrep", [GP, TP * GP], FP8,
                                kind="ExternalInput")
    b2_d = dt("b2", [GP, NG])
    a3_d = dt("a3", [COUT, 1])
    b3_d = dt("b3", [COUT, 1])
    dwT_d = dt("dwT", [GP, E, NG, T])
    pwT_d = dt("pwT", [GP, E, NG, COUT])
    sw1_d = dt("sw1", [GP, NG, RED])
    sw2b_d = dt("sw2b", [RED, NG, GP])
    b2se_d = dt("b2se", [GP, NG])
    rw1_d = dt("rw1", [CIN, RHID])
    rb1_d = dt("rb1", [RHID, 1])
    rw2_d = dt("rw2", [RHID, E])
    rb2_d = dt("rb2", [BL, E])
    sb1_d = dt("sb1", [RED, 1])
    y_d = nc.dram_tensor("y", [BL, COUT, HW], F32, kind="ExternalOutput")

    with tile.TileContext(nc) as tc:
        with (
            tc.tile_pool(name="consts", bufs=1) as cp,
            tc.tile_pool(name="dram", bufs=1, space="DRAM") as dp,
            tc.tile_pool(name="xpad", bufs=1) as xpp,
            tc.tile_pool(name="out2", bufs=1) as o2p,
            tc.tile_pool(name="diag", bufs=7) as dgp,
            tc.tile_pool(name="wscp", bufs=2) as wsp,
            tc.tile_pool(name="outb", bufs=2) as obp,
            tc.tile_pool(name="small", bufs=2) as smp,
            tc.tile_pool(name="ppex", bufs=2, space="PSUM") as ppex,
            tc.tile_pool(name="pse", bufs=1, space="PSUM") as psep,
            tc.tile_pool(name="pdw", bufs=3, space="PSUM") as pdwp,
            tc.tile_pool(name="ppw", bufs=1, space="PSUM") as ppwp,
        ):
            # ---- persistent consts ----
            x_sb = cp.tile([CIN, BL, HW], F32, tag="x_sb")
            x8 = cp.tile([CH, 2, BL, HW], FP8, tag="x8")
            exp8 = cp.tile([CH, 2, HIDP], FP8, tag="exp8")
            b2 = cp.tile([GP, NG], F32, tag="b2")
            a3 = cp.tile([COUT, 1], F32, tag="a3")
            b3 = cp.tile([COUT, 1], F32, tag="b3")
            dwT = cp.tile([GP, E, NG, T], F32, tag="dwT")
            pwT = cp.tile([GP, E, NG, COUT], F32, tag="pwT")
            sw1 = cp.tile([GP, NG, RED], F32, tag="sw1")
            sw2b = cp.tile([RED, NG, GP], F32, tag="sw2b")
            b2se = cp.tile([GP, NG], F32, tag="b2se")
            rw1 = cp.tile([CIN, RHID], F32, tag="rw1")
            rb1 = cp.tile([RHID, 1], F32, tag="rb1")
            rw2 = cp.tile([RHID, E], F32, tag="rw2")
            rb2 = cp.tile([BL, E], F32, tag="rb2")
            sb1 = cp.tile([RED, 1], F32, tag="sb1")
            identrep = cp.tile([GP, TP * GP], FP8, tag="identrep")
            kern = cp.tile([GP, NG, BL, TP], F32, tag="kern")
            pwag = cp.tile([GP, BL, NG, COUT], F32, tag="pwag")
            rw_bc = cp.tile([GP, BL * E], F32, tag="rw_bc")

            # ---- padded x1 tiles: 2 slots x NG chunks, fp8, zeroed once ----
            xp_t = [
                [xpp.tile([GP, XPW], FP8, tag=f"xp{s}g{g}", name=f"xp{s}g{g}")
                 for g in range(NG)]
                for s in range(2)
            ]
            for s in range(2):
                for g in range(NG):
                    nc.gpsimd.memset(xp_t[s][g][:], 0.0)
            # zero taps 25 and 27 stay zero forever; slot 26 holds b2 per iter
            nc.gpsimd.memset(kern[:, :, :, T : T + 1], 0.0)

            out2_t = [o2p.tile([GP, NG, HW], FP8, tag=f"o2{s}", name=f"o2{s}")
                      for s in range(2)]
            s_parts = [cp.tile([GP, 2 * NG], F32, tag=f"sp{b}", name=f"sp{b}")
                       for b in range(BL)]

            def build_diag(eng, dg, b, g, t_lo, t_hi):
                # dg[:, t, :] = diag(64 * kern[:, g, b, t]) for [t_lo, t_hi):
                # (kern_f32 bcast x 64) * packed replicated-identity mask
                n = t_hi - t_lo
                k_ap = kern[:, g, b, t_lo:t_hi]
                k_b = bass.AP(tensor=k_ap.tensor, offset=k_ap.offset,
                              ap=[k_ap.ap[0], [1, n], [0, GP]])
                eng.scalar_tensor_tensor(
                    dg[:, t_lo:t_hi, :], k_b, KSCALE,
                    identrep[:, t_lo * GP : t_hi * GP],
                    op0=ALU.mult, op1=ALU.mult)

            def emit_se_pw(b, slot):
                """SE chain + pointwise matmul + bn3/residual + store for b."""
                s_sum = smp.tile([GP, NG], F32, tag="s_sum", name="s_sum")
                sp10 = s_parts[b][:].rearrange("p (g n) -> p g n", n=2)
                nc.vector.tensor_reduce(s_sum[:], sp10, axis=AX.X, op=ALU.add)
                pz = psep.tile([RED, 1], F32, tag="se", name="pz")
                for g in range(NG):
                    nc.tensor.matmul(pz[:], sw1[:, g],
                                     s_sum[:, g : g + 1],
                                     start=(g == 0), stop=(g == NG - 1))
                zt = smp.tile([RED, 1], F32, tag="zt", name="zt")
                nc.scalar.activation(zt[:], pz[:], AF.Silu, bias=sb1[:],
                                     scale=1.0)
                psc = psep.tile([GP, NG], F32, tag="se", name="psc")
                for g in range(NG):
                    nc.tensor.matmul(psc[:, g : g + 1], sw2b[:, g], zt[:],
                                     start=True, stop=True)
                # sigmoid via tanh (stays in the silu ACT table set):
                # sigmoid(p + b) = 0.5 + 0.5*tanh(0.5*p + 0.5*b); b2se pre-halved
                ut = smp.tile([GP, NG], F32, tag="ut", name="ut")
                nc.vector.scalar_tensor_tensor(ut[:], psc[:], 0.5, b2se[:],
                                               op0=ALU.mult, op1=ALU.add)
                sc = smp.tile([GP, NG], F32, tag="sc", name="sc")
                nc.scalar.activation(sc[:], ut[:], AF.Tanh)
                nc.vector.tensor_scalar(sc[:], sc[:], 0.5, 0.5,
                                        op0=ALU.mult, op1=ALU.add)
                wsc = wsp.tile([GP, NG, COUT], FP8, tag="wsc", name="wsc")
                for g in range(NG):
                    nc.vector.tensor_scalar_mul(wsc[:, g], pwag[:, b, g],
                                                sc[:, g : g + 1])
                # pointwise projection: 2 DoubleRow chunk-pairs + 1 plain fp8
                po = ppwp.tile([COUT, NH, 512], F32, tag="po", name="po")
                wsc_ap = wsc[:]
                o2_ap = out2_t[slot][:]
                for nh in range(NH):
                    for gp_i in range(2):
                        g0 = 2 * gp_i
                        lhsT = bass.AP(
                            tensor=wsc_ap.tensor,
                            offset=wsc_ap.offset + g0 * COUT,
                            ap=[wsc_ap.ap[0], [COUT, 2], [1, COUT]])
                        rhs = bass.AP(
                            tensor=o2_ap.tensor,
                            offset=o2_ap.offset + g0 * HW + nh * NF,
                            ap=[o2_ap.ap[0], [HW, 2], [1, NF]])
                        nc.tensor.matmul(po[:, nh, :NF], lhsT, rhs,
                                         start=(gp_i == 0), stop=False,
                                         perf_mode=DR)
                    nc.tensor.matmul(
                        po[:, nh, :NF], wsc[:, NG - 1],
                        out2_t[slot][:, NG - 1, nh * NF : (nh + 1) * NF],
                        start=False, stop=True)
                ob = obp.tile([COUT, HW], F32, tag="ob", name="ob")
                po_ap = po[:]
                po_v = bass.AP(tensor=po_ap.tensor, offset=po_ap.offset,
                               ap=[po_ap.ap[0], [512, NH], [1, NF]])
                nc.vector.scalar_tensor_tensor(
                    ob[:], po_v, a3[:], x_sb[:, b, :],
                    op0=ALU.mult, op1=ALU.add)
                nc.vector.tensor_scalar_add(ob[:], ob[:], b3[:])
                nc.sync.dma_start(y_d[b], ob[:])

            def emit_body():
                nc.sync.dma_start(x_sb[:], x_d[:])
                nc.sync.dma_start(x8[:], x8_d[:])
                for t_sb, t_d in [
                    (exp8, exp8_d), (b2, b2_d), (a3, a3_d), (b3, b3_d),
                    (dwT, dwT_d), (pwT, pwT_d), (sw1, sw1_d), (sw2b, sw2b_d),
                    (b2se, b2se_d), (rw1, rw1_d), (rb1, rb1_d), (rw2, rw2_d),
                    (rb2, rb2_d), (sb1, sb1_d), (identrep, identrep_d),
                ]:
                    nc.sync.dma_start(t_sb[:], t_d[:])

                # routing: pool -> MLP -> softmax (samples on partitions).
                # exp(v) for v<=0 computed as (1+t)/(1-t) with t=tanh(v/2) so
                # the act table set never leaves the silu family.
                xsum = cp.tile([CIN, BL], F32, tag="xsum", name="xsum")
                nc.vector.tensor_reduce(xsum[:], x_sb[:], axis=AX.X, op=ALU.add)
                ph1 = psep.tile([RHID, BL], F32, tag="se", name="ph1")
                nc.tensor.matmul(ph1[:], rw1[:], xsum[:], start=True, stop=True)
                hdn = cp.tile([RHID, BL], F32, tag="hdn", name="hdn")
                nc.scalar.activation(hdn[:], ph1[:], AF.Relu, bias=rb1[:], scale=1.0)
                pl2 = psep.tile([BL, E], F32, tag="se", name="pl2")
                nc.tensor.matmul(pl2[:], hdn[:], rw2[:], start=True, stop=True)
                lt = cp.tile([BL, E], F32, tag="lt", name="lt")
                nc.vector.tensor_add(lt[:], pl2[:], rb2[:])
                mx = cp.tile([BL, 1], F32, tag="mx", name="mx")
                nc.vector.reduce_max(mx[:], lt[:], axis=AX.X)
                nc.vector.tensor_scalar_sub(lt[:], lt[:], mx[:])
                th = cp.tile([BL, E], F32, tag="th", name="th")
                nc.scalar.activation(th[:], lt[:], AF.Tanh, bias=0.0, scale=0.5)
                el = cp.tile([BL, E], F32, tag="el", name="el")
                den = cp.tile([BL, E], F32, tag="den", name="den")
                nc.vector.tensor_scalar(el[:], th[:], 1.0, 1.0,
                                        op0=ALU.mult, op1=ALU.add)
                nc.vector.tensor_scalar(den[:], th[:], -1.0, 1.0,
                                        op0=ALU.mult, op1=ALU.add)
                nc.vector.reciprocal(den[:], den[:])
                nc.vector.tensor_mul(el[:], el[:], den[:])
                es = cp.tile([BL, 1], F32, tag="es", name="es")
                nc.vector.reduce_sum(es[:], el[:], axis=AX.X)
                einv = cp.tile([BL, 1], F32, tag="einv", name="einv")
                nc.vector.reciprocal(einv[:], es[:])
                rwT = cp.tile([BL, E], F32, tag="rwT", name="rwT")
                nc.vector.tensor_scalar_mul(rwT[:], el[:], einv[:])
                # broadcast rw to all 128 partitions via DRAM bounce
                rw_dram = dp.tile([BL, E], F32, tag="rwd", name="rwd")
                nc.sync.dma_start(rw_dram[:], rwT[:])
                rwd_ap = rw_dram[:]
                bcast_src = bass.AP(
                    tensor=rwd_ap.tensor, offset=rwd_ap.offset,
                    ap=[[0, GP], [1, BL * E]],
                )
                nc.sync.dma_start(rw_bc[:], bcast_src)

                # expert-weight aggregation (runtime routing weights)
                for b in range(BL):
                    kv = kern[:, :, b, 0:T]
                    pv = pwag[:, b]
                    for e in range(E):
                        s_ap = rw_bc[:, E * b + e : E * b + e + 1]
                        if e == 0:
                            nc.vector.tensor_scalar_mul(kv, dwT[:, e], s_ap)
                            nc.vector.tensor_scalar_mul(pv, pwT[:, e], s_ap)
                        else:
                            nc.vector.scalar_tensor_tensor(
                                kv, dwT[:, e], s_ap, kv, op0=ALU.mult, op1=ALU.add)
                            nc.vector.scalar_tensor_tensor(
                                pv, pwT[:, e], s_ap, pv, op0=ALU.mult, op1=ALU.add)

                def emit_expand(b, g):
                    # expand conv (fp8 DR over channel halves + ones channel
                    # carrying 8*b1) + silu(psum/8) into padded fp8 layout
                    slot = b % 2
                    lhsT = bass.AP(
                        tensor=exp8[:].tensor,
                        offset=exp8[:].offset + g * GP,
                        ap=[exp8[:].ap[0], [HIDP, 2], [1, GP]])
                    for nh in range(NH):
                        pex = ppex.tile([GP, NF], F32, tag="pex", name="pex")
                        x8v = x8[:]
                        rhs = bass.AP(
                            tensor=x8v.tensor,
                            offset=x8v.offset + b * HW + nh * NF,
                            ap=[x8v.ap[0], [BL * HW, 2], [1, NF]])
                        nc.tensor.matmul(pex[:], lhsT, rhs,
                                         start=True, stop=True, perf_mode=DR)
                        xpv = xp_t[slot][g][:, 68 + nh * 448 : 68 + nh * 448 + 434]
                        xpo = bass.AP(tensor=xpv.tensor, offset=xpv.offset,
                                      ap=[xpv.ap[0], [PW, RH], [1, W]])
                        nc.scalar.activation(xpo, pex[:], AF.Silu,
                                             bias=0.0, scale=1.0 / ESCALE)

                dgs_hold = [None]
                for b in range(BL):
                    slot = b % 2
                    if b > 0:
                        emit_se_pw(b - 1, 1 - slot)
                    for g in range(NG):
                        emit_expand(b, g)
                    # fp8 diag slabs for this sample
                    if "nodiag" in ablate and b > 0:
                        dgs = dgs_hold[0]
                    else:
                        dgs = [dgp.tile([GP, TP, GP], FP8, tag="dg", name="dg")
                               for g in range(NG)]
                        for eng_name, g, t_lo, t_hi in DIAG_SPLIT:
                            eng = nc.vector if eng_name == "dve" else nc.gpsimd
                            build_diag(eng, dgs[g], b, g, t_lo, t_hi)
                        dgs_hold[0] = dgs
                    # depthwise conv: fp8 DoubleRow tap-pair matmuls per chunk;
                    # one stationary per (g, pair) shared by both halves
                    for g in range(NG):
                        pdw = [pdwp.tile([GP, 448], F32, tag="pdw", name="pdw")
                               for _ in range(NH)]
                        pairs = DW_PAIRS
                        if "half_pairs" in ablate:
                            pairs = DW_PAIRS[:7]
                        dga = dgs[g][:]
                        for pi, (t0, t1) in enumerate(pairs):
                            stride = 2 if t1 >= T else _tap_off(t1) - _tap_off(t0)
                            lhsT = bass.AP(
                                tensor=dga.tensor, offset=dga.offset + t0 * GP,
                                ap=[dga.ap[0], [(t1 - t0) * GP, 2], [1, GP]])
                            for nh in range(NH):
                                off = 448 * nh + _tap_off(t0)
                                rv = xp_t[slot][g][:, off : off + 448]
                                rhs = bass.AP(tensor=rv.tensor, offset=rv.offset,
                                              ap=[rv.ap[0], [stride, 2], [1, 448]])
                                nc.tensor.matmul(pdw[nh][:], lhsT, rhs,
                                                 start=(pi == 0),
                                                 stop=(pi == len(pairs) - 1),
                                                 perf_mode=DR)
                        for nh in range(NH):
                            pv = pdw[nh][:, 2:436]
                            pvo = bass.AP(tensor=pv.tensor, offset=pv.offset,
                                          ap=[pv.ap[0], [PW, RH], [1, W]])
                            nc.scalar.activation(
                                out2_t[slot][:, g, nh * NF : (nh + 1) * NF],
                                pvo, AF.Silu, bias=b2[:, g : g + 1],
                                scale=1.0 / KSCALE,
                                accum_out=s_parts[b][:, 2 * g + nh
                                                     : 2 * g + nh + 1])
                emit_se_pw(BL - 1, (BL - 1) % 2)

            loop_ctx = (tc.For_i(0, reps, 1, hint_engines=(mybir.EngineType.PE,))
                        if reps > 1 else contextlib.nullcontext())
            with loop_ctx:
                emit_body()

    nc.compile()
    return nc


_NC = None


def _get_nc():
    global _NC
    if _NC is None:
        _NC = _build_program()
    return _NC


def _prep_maps(x, r_w1, r_b1, r_w2, r_b2, exp_w,
               bn1_g, bn1_b, bn1_m, bn1_v, dw_w,
               bn2_g, bn2_b, bn2_m, bn2_v,
               se_w1, se_b1, se_w2, se_b2, pw_w,
               bn3_g, bn3_b, bn3_m, bn3_v):
    f = np.float32
    f8dt = mybir.dt.np(FP8)
    x = np.asarray(x, f).reshape(B, CIN, HW)

    def fold_bn(g, bvec, m, v):
        a = np.asarray(g, f) / np.sqrt(np.asarray(v, f) + EPS)
        return a, np.asarray(bvec, f) - np.asarray(m, f) * a

    a1v, b1v = fold_bn(bn1_g, bn1_b, bn1_m, bn1_v)
    a2v, b2v = fold_bn(bn2_g, bn2_b, bn2_m, bn2_v)
    a3v, b3v = fold_bn(bn3_g, bn3_b, bn3_m, bn3_v)
    a3v = a3v / np.float32(PSCALE)

    def chunk(v):  # [HID] -> [GP, NG] padded
        vp = np.concatenate([np.asarray(v, f), np.zeros(HIDP - HID, f)])
        return vp.reshape(NG, GP).T.copy()

    # expand weights with a1 folded in, x8 scale, + ones channel carrying b1
    expT = np.zeros((CIN, HIDP), f)
    expT[:, :HID] = (np.asarray(exp_w, f) * a1v[:, None]).T
    b1p = np.concatenate([b1v, np.zeros(HIDP - HID, f)])
    exp8 = np.zeros((CH, 2, HIDP), f)
    exp8[: CIN // 2, 0] = expT[: CIN // 2]
    exp8[: CIN // 2, 1] = expT[CIN // 2 :]
    exp8[CIN // 2, 0] = b1p
    exp8 = (exp8 * np.float32(ESCALE)).astype(f8dt)

    # depthwise kernels with a2 folded in (per channel)
    dwf = np.asarray(dw_w, f).reshape(E, HID, T) * a2v[None, :, None]
    dwT = np.zeros((GP, E, NG, T), f)
    pwT = np.zeros((GP, E, NG, COUT), f)
    sw1 = np.zeros((GP, NG, RED), f)
    sw2b = np.zeros((RED, NG, GP), f)
    b2se = np.zeros((GP, NG), f)
    for g in range(NG):
        n = min(GP, HID - g * GP)
        cs = slice(g * GP, g * GP + n)
        dwT[:n, :, g, :] = dwf[:, cs, :].transpose(1, 0, 2)
        pwT[:n, :, g, :] = np.asarray(pw_w, f)[:, :, cs].transpose(2, 0, 1) \
            * np.float32(PSCALE)
        sw1[:n, g, :] = (np.asarray(se_w1, f)[:, cs] / HW).T
        sw2b[:, g, :n] = np.asarray(se_w2, f)[cs, :].T
        b2se[:n, g] = np.asarray(se_b2, f)[cs] / 2

    common = dict(
        exp8=exp8,
        b2=chunk(b2v),
        a3=a3v.reshape(COUT, 1), b3=b3v.reshape(COUT, 1),
        dwT=dwT, pwT=pwT, sw1=sw1, sw2b=sw2b, b2se=b2se,
        rw1=(np.asarray(r_w1, f).T / HW).copy(),
        rb1=np.asarray(r_b1, f).reshape(RHID, 1),
        rw2=np.asarray(r_w2, f).T.copy(),
        rb2=np.tile(np.asarray(r_b2, f), (BL, 1)),
        sb1=np.asarray(se_b1, f).reshape(RED, 1),
        identrep=np.tile(np.eye(GP, dtype=f), (1, TP)).astype(f8dt),
    )
    out = []
    for c in range(NCORES):
        xs = np.ascontiguousarray(x[c * BL : (c + 1) * BL].transpose(1, 0, 2))
        x8 = np.zeros((CH, 2, BL, HW), f)
        x8[: CIN // 2, 0] = xs[: CIN // 2]
        x8[: CIN // 2, 1] = xs[CIN // 2 :]
        x8[CIN // 2, 0] = 1.0
        out.append(dict(common, x=xs, x8=x8.astype(f8dt)))
    return out


def kernel(**inputs):
    from concourse.bass_utils import run_bass_kernel_spmd

    nc = _get_nc()
    in_maps = _prep_maps(**inputs)
    res = run_bass_kernel_spmd(nc, in_maps, core_ids=list(range(NCORES)))
    y = np.concatenate([res.results[c]["y"] for c in range(NCORES)], axis=0)
    return y.reshape(B, COUT, H, W).astype(np.float32)


if __name__ == "__main__":
    t0 = time.time()
    nc = _get_nc()
    print(f"build+compile: {time.time()-t0:.1f}s")

